# revision 5
# baseline (speedup 1.0000x reference)
"""Barrel shifter right 64 (zero-fill), batch 2097152, on 8 NeuronCores. v10.

Same engine design as v8 (bf16 device I/O with host per-element dtype
casts, fp8 on-chip compute, DVE predicated stages 2..5, ACT shifted
copies + DVE inverted-mask fixups for stages 1/0, SWDGE cast-DMAs, deep
fixup lags so the DVE<->ACT ping-pong never stalls, 2-tile DMA prefetch).

v10 adds heterogeneous EDGE TILES: the v8 trace showed ~9us ramp and a
~17us serial pipeline-drain on the last 8192-row tile (st -> sh2 -> f1 ->
sh1 -> f0 -> out with nothing left to overlap). Splitting the first tile
into 16/16/32 spans and the last into 32/16/8/8 shrinks both edges to a
few us. Slot rotation is round-robin by TILE INDEX, so every semaphore
formula is unchanged -- only the per-tile views/DRAM indices vary.
"""

import sys

if "/opt/trn_rl_repo" not in sys.path:
    sys.path.insert(0, "/opt/trn_rl_repo")

import numpy as np
import ml_dtypes

BF16 = np.dtype(ml_dtypes.bfloat16)

B_TOTAL = 2097152
NBITS = 64
NCTRL = 6
NCORES = 8
R_FULL = B_TOTAL // NCORES  # 262144 rows per core

P = 128
SPANS = 64                  # max spans per tile (slot buffers sized for this)
FD8 = SPANS * NBITS
SFD = SPANS * NCTRL
NS = 8                      # rotating slots per tile class
OUT_LAG = 6                 # out-DMA of tile m issues alongside in-DMA of m+OUT_LAG
F1_LAG = 2                  # fixup1 of tile n-F1_LAG runs in DVE tile n
F0_LAG = 3                  # fixup0 of tile n-F0_LAG runs in DVE tile n
                            # (shorter than v8/v10 so the last tiles' fixup0 ->
                            #  out-DMA chains finish closer to end of compute)

_built = {}


def _tile_plan(rows):
    units = rows // P  # span-units of 128 rows
    head = [16, 16, 32]
    tail = [32, 32, 32, 16, 16]
    mid_units = units - sum(head) - sum(tail)
    assert mid_units >= 0 and mid_units % SPANS == 0
    sizes = head + [SPANS] * (mid_units // SPANS) + tail
    plan = []
    off = 0
    for sz in sizes:
        assert off % sz == 0, (off, sz)
        plan.append((sz, off // sz))
        off += sz
    assert off == units
    return plan


def build(rows, ns=NS):
    import concourse.bass as bass
    from concourse import mybir

    f32 = mybir.dt.float32
    bf16 = mybir.dt.bfloat16
    fp8 = mybir.dt.float8e4
    i32 = mybir.dt.int32
    i16 = mybir.dt.int16
    i8 = mybir.dt.int8

    plan = _tile_plan(rows)
    nt = len(plan)
    assert nt >= ns >= 5 and OUT_LAG < ns and F0_LAG < ns and F1_LAG < F0_LAG

    nc = bass.Bass()
    data = nc.declare_dram_parameter("data", [rows, NBITS], bf16, isOutput=False)
    shift = nc.declare_dram_parameter("shift", [rows, NCTRL], f32, isOutput=False)
    out = nc.declare_dram_parameter("out", [rows, NBITS], bf16, isOutput=True)

    szs = sorted({sz for sz, _ in plan})
    data_v = {s: data.rearrange("(n p t) k -> n p (t k)", p=P, t=s) for s in szs}
    shift_v = {s: shift.rearrange("(n p t) k -> n p (t k)", p=P, t=s) for s in szs}
    out_v = {s: out.rearrange("(n p t) k -> n p (t k)", p=P, t=s) for s in szs}

    A = [nc.alloc_sbuf_tensor(f"A{j}", [P, FD8], fp8) for j in range(ns)]
    T = [nc.alloc_sbuf_tensor(f"T{j}", [P, FD8], fp8) for j in range(ns)]
    O = [nc.alloc_sbuf_tensor(f"O{j}", [P, FD8], fp8) for j in range(ns)]
    S = [nc.alloc_sbuf_tensor(f"S{j}", [P, SFD], f32) for j in range(ns)]
    MK = [nc.alloc_sbuf_tensor(f"MK{j}", [P, SPANS * 2], f32) for j in range(ns)]
    Z = nc.alloc_sbuf_tensor("Z", [P, SPANS * 8], i32)  # static zeros

    def v32(t, sz):
        return t.ap().bitcast(i32).rearrange("p (t c) -> p t c", c=16)[:, 0:sz]

    def v16(t, sz):
        return t.ap().bitcast(i16).rearrange("p (t c) -> p t c", c=32)[:, 0:sz]

    def v8(t, sz):
        return t.ap().bitcast(i8).rearrange("p (t c) -> p t c", c=64)[:, 0:sz]

    def vf8(t, sz):
        return t.ap().rearrange("p (t c) -> p t c", c=64)[:, 0:sz]

    def smask32(s_t, i, w, sz):
        # stage-i select bit (f32 col 5-i) as nonzero-int32, broadcast to w
        return (
            s_t.ap().bitcast(i32)
            .rearrange("p (t j) -> p t j", j=NCTRL)[:, 0:sz, 5 - i:6 - i]
            .broadcast_to([P, sz, w])
        )

    def invmask(r, col, w, sz):
        # materialized inverted mask (1.0-bit): col 0 = stage1, col 1 = stage0
        return (
            MK[r].ap().bitcast(i32)
            .rearrange("p (t c) -> p t c", c=2)[:, 0:sz, col:col + 1]
            .broadcast_to([P, sz, w])
        )

    z32 = Z.ap().rearrange("p (t c) -> p t c", c=8)

    from contextlib import ExitStack

    with ExitStack() as stack:
        block = stack.enter_context(nc.Block())
        s_din = [stack.enter_context(nc.semaphore(f"s_din{j}")) for j in range(ns)]
        s_do = [stack.enter_context(nc.semaphore(f"s_do{j}")) for j in range(ns)]
        s_st = stack.enter_context(nc.semaphore("s_st"))    # DVE st2..5 done
        s_sh2 = stack.enter_context(nc.semaphore("s_sh2"))  # ACT maskgen+sh2 done
        s_f1 = stack.enter_context(nc.semaphore("s_f1"))    # DVE fixup1 done
        s_sh1 = stack.enter_context(nc.semaphore("s_sh1"))  # ACT sh1 done
        s_fix = stack.enter_context(nc.semaphore("s_fix"))  # DVE fixup0 done

        @block.gpsimd
        def _(g):
            for n, (sz, vi) in enumerate(plan):
                if n >= ns:
                    # A slot reusable once fixup1 (last reader) of n-ns done
                    g.wait_ge(s_f1, n - ns + 1)
                g.dma_start(
                    out=A[n % ns].ap()[:, 0:sz * NBITS], in_=data_v[sz][vi]
                ).then_inc(s_din[n % ns], 16)
                m = n - OUT_LAG
                if m >= 0:
                    msz, mvi = plan[m]
                    g.wait_ge(s_fix, m + 1)
                    g.dma_start(
                        out=out_v[msz][mvi],
                        in_=O[m % ns].ap()[:, 0:msz * NBITS],
                    ).then_inc(s_do[m % ns], 16)
            for m in range(nt - OUT_LAG, nt):
                msz, mvi = plan[m]
                g.wait_ge(s_fix, m + 1)
                g.dma_start(
                    out=out_v[msz][mvi], in_=O[m % ns].ap()[:, 0:msz * NBITS]
                ).then_inc(s_do[m % ns], 16)

        @block.sync
        def _(sp):
            for n, (sz, vi) in enumerate(plan):
                if n >= ns:
                    # S slot: last reader is ACT's maskgen (before sh2) of n-ns
                    sp.wait_ge(s_sh2, n - ns + 1)
                sp.dma_start(
                    out=S[n % ns].ap()[:, 0:sz * NCTRL], in_=shift_v[sz][vi]
                ).then_inc(s_din[n % ns], 16)

        def fixup1(v, m):
            sz, r = plan[m][0], m % ns
            v.wait_ge(s_sh2, m + 1)
            v.memset(v16(T[r], sz)[:, :, 0:1], 0)  # lanes 0-1
            v.copy_predicated(
                v32(T[r], sz), invmask(r, 0, 16, sz), v32(A[r], sz)
            ).then_inc(s_f1, 1)

        def fixup0(v, m):
            sz, r = plan[m][0], m % ns
            v.wait_ge(s_sh1, m + 1)
            v.memset(v8(O[r], sz)[:, :, 0:1], 0)   # lane 0
            v.copy_predicated(
                v32(O[r], sz), invmask(r, 1, 16, sz), v32(T[r], sz)
            ).then_inc(s_fix, 1)

        @block.vector
        def _(v):
            v.memset(Z.ap(), 0)
            for n, (sz, vi) in enumerate(plan):
                r = n % ns
                v.wait_ge(s_din[r], 32 * (n // ns + 1))
                a32 = v32(A[r], sz)
                # stages 2..5 (sa = 1,2,4,8 int32) in place, reversed
                for i in range(2, 6):
                    w = (1 << i) // 4
                    v.copy_predicated(
                        a32[:, :, w:16][:, :, ::-1],
                        smask32(S[r], i, 16 - w, sz),
                        a32[:, :, 0:16 - w][:, :, ::-1],
                    )
                    ins = v.copy_predicated(
                        a32[:, :, 0:w], smask32(S[r], i, w, sz),
                        z32[:, 0:sz, 0:w],
                    )
                    if i == 5:
                        ins.then_inc(s_st, 1)
                if n >= F1_LAG:
                    fixup1(v, n - F1_LAG)
                if n >= F0_LAG:
                    fixup0(v, n - F0_LAG)
            for m in range(nt - F1_LAG, nt):
                fixup1(v, m)
            for m in range(nt - F0_LAG, nt):
                fixup0(v, m)

        def _sh1(s, m):
            sz, r = plan[m][0], m % ns
            s.wait_ge(s_f1, m + 1)
            if m >= ns:
                # O slot reusable once out-DMA of m-ns drained
                s.wait_ge(s_do[r], 16 * (m // ns))
            s.copy(
                vf8(O[r], sz)[:, :, 1:64], vf8(T[r], sz)[:, :, 0:63]
            ).then_inc(s_sh1, 1)

        @block.scalar
        def _(s):
            for n, (sz, vi) in enumerate(plan):
                r = n % ns
                s.wait_ge(s_st, n + 1)
                if n >= ns:
                    # T and MK slots reusable once fixup0 of n-ns done
                    s.wait_ge(s_fix, n - ns + 1)
                # inverted masks for stages 1 and 0: 1.0 - bit
                st3 = S[r].ap().rearrange("p (t j) -> p t j", j=NCTRL)
                s.activation(
                    MK[r].ap().rearrange("p (t c) -> p t c", c=2)[:, 0:sz],
                    st3[:, 0:sz, 4:6],
                    mybir.ActivationFunctionType.Identity,
                    bias=1.0,
                    scale=-1.0,
                )
                s.copy(
                    vf8(T[r], sz)[:, :, 2:64], vf8(A[r], sz)[:, :, 0:62]
                ).then_inc(s_sh2, 1)
                if n >= F1_LAG:
                    _sh1(s, n - F1_LAG)
            for m in range(nt - F1_LAG, nt):
                _sh1(s, m)

    return nc


def _get(rows):
    if rows not in _built:
        _built[rows] = build(rows)
    return _built[rows]


def run_cores(data, shift, rows, trace=False):
    from concourse.bass_utils import run_bass_kernel_spmd

    nc = _get(rows)
    ncores = data.shape[0] // rows
    data = np.ascontiguousarray(data).astype(BF16)
    in_maps = [
        {
            "data": np.ascontiguousarray(data[i * rows:(i + 1) * rows]),
            "shift": np.ascontiguousarray(shift[i * rows:(i + 1) * rows]),
        }
        for i in range(ncores)
    ]
    res = run_bass_kernel_spmd(nc, in_maps, list(range(ncores)), trace=trace)
    full = np.concatenate([res.results[i]["out"] for i in range(ncores)], axis=0)
    return full, res


def kernel(data, shift):
    data = np.ascontiguousarray(np.asarray(data), dtype=np.float32)
    shift = np.ascontiguousarray(np.asarray(shift), dtype=np.float32)
    full, _ = run_cores(data, shift, R_FULL)
    return full.astype(np.float32)


# revision 6
# speedup vs baseline: 1.1888x; 1.1888x over previous
"""Barrel shifter right 64 (zero-fill), batch 2097152, on 8 NeuronCores. v10.

Same engine design as v8 (bf16 device I/O with host per-element dtype
casts, fp8 on-chip compute, DVE predicated stages 2..5, ACT shifted
copies + DVE inverted-mask fixups for stages 1/0, SWDGE cast-DMAs, deep
fixup lags so the DVE<->ACT ping-pong never stalls, 2-tile DMA prefetch).

v10 adds heterogeneous EDGE TILES: the v8 trace showed ~9us ramp and a
~17us serial pipeline-drain on the last 8192-row tile (st -> sh2 -> f1 ->
sh1 -> f0 -> out with nothing left to overlap). Splitting the first tile
into 16/16/32 spans and the last into 32/16/8/8 shrinks both edges to a
few us. Slot rotation is round-robin by TILE INDEX, so every semaphore
formula is unchanged -- only the per-tile views/DRAM indices vary.
"""

import sys

if "/opt/trn_rl_repo" not in sys.path:
    sys.path.insert(0, "/opt/trn_rl_repo")

import numpy as np
import ml_dtypes

BF16 = np.dtype(ml_dtypes.bfloat16)

B_TOTAL = 2097152
NBITS = 64
NCTRL = 6
NCORES = 8
R_FULL = B_TOTAL // NCORES  # 262144 rows per core

P = 128
SPANS = 64                  # max spans per tile (slot buffers sized for this)
FD8 = SPANS * NBITS
SFD = SPANS * NCTRL
NS = 8                      # rotating slots per tile class
OUT_LAG = 6                 # out-DMA of tile m issues alongside in-DMA of m+OUT_LAG
F1_LAG = 2                  # fixup1 of tile n-F1_LAG runs in DVE tile n
F0_LAG = 4                  # fixup0 of tile n-F0_LAG runs in DVE tile n

_built = {}


def _tile_plan(rows):
    units = rows // P  # span-units of 128 rows
    head = [16, 16, 32]
    tail = [32, 16, 8, 8]
    mid_units = units - sum(head) - sum(tail)
    assert mid_units >= 0 and mid_units % SPANS == 0
    sizes = head + [SPANS] * (mid_units // SPANS) + tail
    plan = []
    off = 0
    for sz in sizes:
        assert off % sz == 0, (off, sz)
        plan.append((sz, off // sz))
        off += sz
    assert off == units
    return plan


def build(rows, ns=NS):
    import concourse.bass as bass
    from concourse import mybir

    f32 = mybir.dt.float32
    bf16 = mybir.dt.bfloat16
    fp8 = mybir.dt.float8e4
    i32 = mybir.dt.int32
    i16 = mybir.dt.int16
    i8 = mybir.dt.int8

    plan = _tile_plan(rows)
    nt = len(plan)
    assert nt >= ns >= 5 and OUT_LAG < ns and F0_LAG < ns and F1_LAG < F0_LAG

    nc = bass.Bass()
    data = nc.declare_dram_parameter("data", [rows, NBITS], bf16, isOutput=False)
    shift = nc.declare_dram_parameter("shift", [rows, NCTRL], f32, isOutput=False)
    out = nc.declare_dram_parameter("out", [rows, NBITS], bf16, isOutput=True)

    szs = sorted({sz for sz, _ in plan})
    data_v = {s: data.rearrange("(n p t) k -> n p (t k)", p=P, t=s) for s in szs}
    shift_v = {s: shift.rearrange("(n p t) k -> n p (t k)", p=P, t=s) for s in szs}
    out_v = {s: out.rearrange("(n p t) k -> n p (t k)", p=P, t=s) for s in szs}

    A = [nc.alloc_sbuf_tensor(f"A{j}", [P, FD8], fp8) for j in range(ns)]
    T = [nc.alloc_sbuf_tensor(f"T{j}", [P, FD8], fp8) for j in range(ns)]
    O = [nc.alloc_sbuf_tensor(f"O{j}", [P, FD8], fp8) for j in range(ns)]
    S = [nc.alloc_sbuf_tensor(f"S{j}", [P, SFD], f32) for j in range(ns)]
    MK = [nc.alloc_sbuf_tensor(f"MK{j}", [P, SPANS * 2], f32) for j in range(ns)]
    Z = nc.alloc_sbuf_tensor("Z", [P, SPANS * 8], i32)  # static zeros

    def v32(t, sz):
        return t.ap().bitcast(i32).rearrange("p (t c) -> p t c", c=16)[:, 0:sz]

    def v16(t, sz):
        return t.ap().bitcast(i16).rearrange("p (t c) -> p t c", c=32)[:, 0:sz]

    def v8(t, sz):
        return t.ap().bitcast(i8).rearrange("p (t c) -> p t c", c=64)[:, 0:sz]

    def vf8(t, sz):
        return t.ap().rearrange("p (t c) -> p t c", c=64)[:, 0:sz]

    def smask32(s_t, i, w, sz):
        # stage-i select bit (f32 col 5-i) as nonzero-int32, broadcast to w
        return (
            s_t.ap().bitcast(i32)
            .rearrange("p (t j) -> p t j", j=NCTRL)[:, 0:sz, 5 - i:6 - i]
            .broadcast_to([P, sz, w])
        )

    def invmask(r, col, w, sz):
        # materialized inverted mask (1.0-bit): col 0 = stage1, col 1 = stage0
        return (
            MK[r].ap().bitcast(i32)
            .rearrange("p (t c) -> p t c", c=2)[:, 0:sz, col:col + 1]
            .broadcast_to([P, sz, w])
        )

    z32 = Z.ap().rearrange("p (t c) -> p t c", c=8)

    from contextlib import ExitStack

    with ExitStack() as stack:
        block = stack.enter_context(nc.Block())
        s_din = [stack.enter_context(nc.semaphore(f"s_din{j}")) for j in range(ns)]
        s_do = [stack.enter_context(nc.semaphore(f"s_do{j}")) for j in range(ns)]
        s_st = stack.enter_context(nc.semaphore("s_st"))    # DVE st2..5 done
        s_sh2 = stack.enter_context(nc.semaphore("s_sh2"))  # ACT maskgen+sh2 done
        s_f1 = stack.enter_context(nc.semaphore("s_f1"))    # DVE fixup1 done
        s_sh1 = stack.enter_context(nc.semaphore("s_sh1"))  # ACT sh1 done
        s_fix = stack.enter_context(nc.semaphore("s_fix"))  # DVE fixup0 done

        @block.gpsimd
        def _(g):
            for n, (sz, vi) in enumerate(plan):
                if n >= ns:
                    # A slot reusable once fixup1 (last reader) of n-ns done
                    g.wait_ge(s_f1, n - ns + 1)
                g.dma_start(
                    out=A[n % ns].ap()[:, 0:sz * NBITS], in_=data_v[sz][vi]
                ).then_inc(s_din[n % ns], 16)
                m = n - OUT_LAG
                if m >= 0:
                    msz, mvi = plan[m]
                    g.wait_ge(s_fix, m + 1)
                    g.dma_start(
                        out=out_v[msz][mvi],
                        in_=O[m % ns].ap()[:, 0:msz * NBITS],
                    ).then_inc(s_do[m % ns], 16)
            for m in range(nt - OUT_LAG, nt):
                msz, mvi = plan[m]
                g.wait_ge(s_fix, m + 1)
                g.dma_start(
                    out=out_v[msz][mvi], in_=O[m % ns].ap()[:, 0:msz * NBITS]
                ).then_inc(s_do[m % ns], 16)

        @block.sync
        def _(sp):
            for n, (sz, vi) in enumerate(plan):
                if n >= ns:
                    # S slot: last reader is ACT's maskgen (before sh2) of n-ns
                    sp.wait_ge(s_sh2, n - ns + 1)
                sp.dma_start(
                    out=S[n % ns].ap()[:, 0:sz * NCTRL], in_=shift_v[sz][vi]
                ).then_inc(s_din[n % ns], 16)

        def fixup1(v, m):
            sz, r = plan[m][0], m % ns
            v.wait_ge(s_sh2, m + 1)
            v.memset(v16(T[r], sz)[:, :, 0:1], 0)  # lanes 0-1
            v.copy_predicated(
                v32(T[r], sz), invmask(r, 0, 16, sz), v32(A[r], sz)
            ).then_inc(s_f1, 1)

        def fixup0(v, m):
            sz, r = plan[m][0], m % ns
            v.wait_ge(s_sh1, m + 1)
            v.memset(v8(O[r], sz)[:, :, 0:1], 0)   # lane 0
            v.copy_predicated(
                v32(O[r], sz), invmask(r, 1, 16, sz), v32(T[r], sz)
            ).then_inc(s_fix, 1)

        @block.vector
        def _(v):
            v.memset(Z.ap(), 0)
            for n, (sz, vi) in enumerate(plan):
                r = n % ns
                v.wait_ge(s_din[r], 32 * (n // ns + 1))
                a32 = v32(A[r], sz)
                # stages 2..5 (sa = 1,2,4,8 int32) in place, reversed
                for i in range(2, 6):
                    w = (1 << i) // 4
                    v.copy_predicated(
                        a32[:, :, w:16][:, :, ::-1],
                        smask32(S[r], i, 16 - w, sz),
                        a32[:, :, 0:16 - w][:, :, ::-1],
                    )
                    ins = v.copy_predicated(
                        a32[:, :, 0:w], smask32(S[r], i, w, sz),
                        z32[:, 0:sz, 0:w],
                    )
                    if i == 5:
                        ins.then_inc(s_st, 1)
                if n >= F1_LAG:
                    fixup1(v, n - F1_LAG)
                if n >= F0_LAG:
                    fixup0(v, n - F0_LAG)
            for m in range(nt - F1_LAG, nt):
                fixup1(v, m)
            for m in range(nt - F0_LAG, nt):
                fixup0(v, m)

        def _sh1(s, m):
            sz, r = plan[m][0], m % ns
            s.wait_ge(s_f1, m + 1)
            if m >= ns:
                # O slot reusable once out-DMA of m-ns drained
                s.wait_ge(s_do[r], 16 * (m // ns))
            s.copy(
                vf8(O[r], sz)[:, :, 1:64], vf8(T[r], sz)[:, :, 0:63]
            ).then_inc(s_sh1, 1)

        @block.scalar
        def _(s):
            for n, (sz, vi) in enumerate(plan):
                r = n % ns
                s.wait_ge(s_st, n + 1)
                if n >= ns:
                    # T and MK slots reusable once fixup0 of n-ns done
                    s.wait_ge(s_fix, n - ns + 1)
                # inverted masks for stages 1 and 0: 1.0 - bit
                st3 = S[r].ap().rearrange("p (t j) -> p t j", j=NCTRL)
                s.activation(
                    MK[r].ap().rearrange("p (t c) -> p t c", c=2)[:, 0:sz],
                    st3[:, 0:sz, 4:6],
                    mybir.ActivationFunctionType.Identity,
                    bias=1.0,
                    scale=-1.0,
                )
                s.copy(
                    vf8(T[r], sz)[:, :, 2:64], vf8(A[r], sz)[:, :, 0:62]
                ).then_inc(s_sh2, 1)
                if n >= F1_LAG:
                    _sh1(s, n - F1_LAG)
            for m in range(nt - F1_LAG, nt):
                _sh1(s, m)

    return nc


def _get(rows):
    if rows not in _built:
        _built[rows] = build(rows)
    return _built[rows]


def run_cores(data, shift, rows, trace=False):
    from concourse.bass_utils import run_bass_kernel_spmd

    nc = _get(rows)
    ncores = data.shape[0] // rows
    data = np.ascontiguousarray(data).astype(BF16)
    in_maps = [
        {
            "data": np.ascontiguousarray(data[i * rows:(i + 1) * rows]),
            "shift": np.ascontiguousarray(shift[i * rows:(i + 1) * rows]),
        }
        for i in range(ncores)
    ]
    res = run_bass_kernel_spmd(nc, in_maps, list(range(ncores)), trace=trace)
    full = np.concatenate([res.results[i]["out"] for i in range(ncores)], axis=0)
    return full, res


def kernel(data, shift):
    data = np.ascontiguousarray(np.asarray(data), dtype=np.float32)
    shift = np.ascontiguousarray(np.asarray(shift), dtype=np.float32)
    full, _ = run_cores(data, shift, R_FULL)
    return full.astype(np.float32)
